# revision 9
# baseline (speedup 1.0000x reference)
"""Trainium2 Bass kernel for an attention block (B=16, C=512, T=2048).

reference:
  q = wq@x + bq; k = wk@x + bk; v = wv@x + bv          (conv1x1 per sample)
  attn = softmax(q^T k over s); out = v @ attn^T
  result = gamma * out + x

Sharding: data-parallel over batch across 8 NeuronCores (2 samples/core),
weights replicated.

Device algorithm (per sample):
  - host folds gamma into wv, and gamma*bv + x into the residual xg
    (softmax rows sum to 1, so the v-bias is a per-channel constant);
    bk is dropped (a per-t constant in scores cancels in softmax over s).
  - q/k/scores path in fp16 (1 PE cycle/row, 11-bit mantissa incl
    implicit); v/softmax-weights path in bf16 (range: exp(S) up to e^64);
    PSUM accumulation always fp32.
  - v^T[s,o] tiles via matmul(lhsT=x_fp16[c,s], rhs=(gamma*wv)^T[c,o])
  - q[d,t], k[d,s] via matmul(lhsT=wq^T/wk^T, rhs=x_fp16); bias only on q
  - per 512-wide t-chunk, for each 128-wide s-chunk (sw pipelined by 2,
    next chunk's first two S^T/exp pairs pre-emitted so the PE never
    starves across chunk boundaries -> HAM stays at K=8/8):
      S^T[s,t] = matmul(lhsT=k[:,s], rhs=q[:,t])      (fp16, K=64, N=512)
      E = exp(S^T)  (ACT, PSUM->SBUF bf16; no max-subtraction: |S|<~64)
      den += matmul(lhsT=ones128, rhs=E)              (bf16; sum over s,
                                                       broadcast on parts)
      out0[c,t] += matmul(lhsT=v^T[s,c], rhs=E)       (bf16; 4 c-chunks)
    then: out0 -> SBUF (ACT, frees PSUM), r = recip(den) (DVE),
          result = out0 * r + xg (DVE mul+add) -> DMA out
"""
import numpy as np
import ml_dtypes
import concourse.bass as bass
import concourse.bacc as bacc
import concourse.tile as tile
from concourse import mybir
from concourse.bass_utils import run_bass_kernel_spmd

F32 = mybir.dt.float32
FP16 = mybir.dt.float16
BF16 = mybir.dt.bfloat16
AF = mybir.ActivationFunctionType

B, C, T, D = 16, 512, 2048, 64
NCORES = 8
BPC = B // NCORES          # samples per core
CCH = C // 128             # 4 channel chunks
TW = 512                   # t tile width (matmul free dim)
TCH = T // TW              # 4 t chunks
SCH = T // 128             # 16 s chunks

PROFILE = False            # set True before calling kernel() to capture HW time
LAST_EXEC_NS = None
_CACHE = {}


def _round_fp32r(a: np.ndarray) -> np.ndarray:
    """Round fp32 to fp32r precision (11 explicit mantissa bits, RNE)."""
    u = np.ascontiguousarray(a, dtype=np.float32).view(np.uint32)
    lsb = (u >> 12) & 1
    rounded = u + np.uint32(0x7FF) + lsb
    return (rounded & np.uint32(0xFFFFF000)).astype(np.uint32).view(np.float32)


def _build():
    nc = bacc.Bacc("TRN2", target_bir_lowering=False, debug=False,
                   enable_asserts=False)
    xd = nc.dram_tensor("x", [BPC, C, T], FP16, kind="ExternalInput").ap()
    xgd = nc.dram_tensor("xg", [BPC, C, T], F32, kind="ExternalInput").ap()
    wkqT = nc.dram_tensor("wkqT", [C, 2 * D], FP16, kind="ExternalInput").ap()
    wvT = nc.dram_tensor("wvT", [C, C], FP16, kind="ExternalInput").ap()
    bqd = nc.dram_tensor("bq", [D, 1], F32, kind="ExternalInput").ap()
    onesd = nc.dram_tensor("ones", [128, 128], BF16, kind="ExternalInput").ap()
    outd = nc.dram_tensor("out", [BPC, C, T], F32, kind="ExternalOutput").ap()

    with tile.TileContext(nc) as tc:
        with tc.tile_pool(name="const", bufs=1) as constp, \
             tc.tile_pool(name="xp", bufs=2) as xp, \
             tc.tile_pool(name="vtp", bufs=1) as vtp, \
             tc.tile_pool(name="qkp", bufs=1) as qkp, \
             tc.tile_pool(name="etp", bufs=1) as etp, \
             tc.tile_pool(name="finp", bufs=1) as finp, \
             tc.tile_pool(name="ps", bufs=1, space="PSUM") as ps:

            # ---- tiles for x (fp16, feeds q/k and v^T matmuls) ----
            x_sb_all = []
            for b in range(BPC):
                x_sb_all.append([xp.tile([128, T], FP16, name=f"x_{b}_{cc}",
                                         tag=f"x{cc}") for cc in range(CCH)])

            def load_x(b, q4, split=1):
                for h in range(split):
                    w = TW // split
                    qsl = slice(q4 * TW + h * w, q4 * TW + (h + 1) * w)
                    for cc in range(CCH):
                        csl = slice(cc * 128, (cc + 1) * 128)
                        nc.sync.dma_start(out=x_sb_all[b][cc][:, qsl],
                                          in_=xd[b, csl, qsl])

            # first x quarter (vT/qk critical path), then weights, then rest
            load_x(0, 0, split=2)
            wv_sb, wq_sb = [], []
            for cc in range(CCH):
                csl = slice(cc * 128, (cc + 1) * 128)
                t_wv = constp.tile([128, C], FP16, name=f"wv{cc}")
                nc.sync.dma_start(out=t_wv, in_=wvT[csl, :])
                wv_sb.append(t_wv)
                t_wkq = constp.tile([128, 2 * D], FP16, name=f"wkq{cc}")
                nc.sync.dma_start(out=t_wkq, in_=wkqT[csl, :])
                wq_sb.append(t_wkq)
            ones = constp.tile([128, 128], BF16)
            nc.sync.dma_start(out=ones, in_=onesd)
            bq_full = constp.tile([128, 1], F32)
            nc.sync.dma_start(out=bq_full[D:2 * D, :], in_=bqd)
            bq_hi = bq_full[D:2 * D, :]
            for q4 in range(1, 4):
                load_x(0, q4)

            for b in range(BPC):
                x_sb = x_sb_all[b]
                if b > 0:
                    for q4 in range(4):
                        load_x(b, q4)

                # ---- v^T tiles (bf16): vt[sc][s=128, o=512] ----
                vt_sb = []
                for sc in range(SCH):
                    vps = ps.tile([128, TW], F32, name=f"vps_{b}_{sc}",
                                  tag=f"o{sc % 2}")
                    for cc in range(CCH):
                        nc.tensor.matmul(
                            vps[:],
                            x_sb[cc][:, sc * 128:(sc + 1) * 128],
                            wv_sb[cc][:],
                            start=(cc == 0), stop=(cc == CCH - 1))
                    t_vt = vtp.tile([128, C], BF16, name=f"vt_{b}_{sc}",
                                    tag=f"vt{sc}")
                    nc.vector.tensor_copy(out=t_vt[:], in_=vps[:])
                    vt_sb.append(t_vt)

                # ---- q, k via one M=128 matmul (k rows 0:64, q rows
                #      64:128); q then shifted to partitions 0:64 by DMA so
                #      the S^T matmul operands share a partition range ----
                q_hi = qkp.tile([128, T], FP16, name=f"qh_{b}", tag="qh")
                q_sb = qkp.tile([D, T], FP16, name=f"q_{b}", tag="q")
                k_sb = qkp.tile([D, T], FP16, name=f"k_{b}", tag="k")
                for tc_i in range(TCH):
                    tsl = slice(tc_i * TW, (tc_i + 1) * TW)
                    qps = ps.tile([128, TW], F32, name=f"qps_{b}_{tc_i}",
                                  tag=f"o{2 + tc_i % 2}")
                    for cc in range(CCH):
                        nc.tensor.matmul(qps[:], wq_sb[cc][:],
                                         x_sb[cc][:, tsl],
                                         start=(cc == 0), stop=(cc == CCH - 1))
                    nc.vector.tensor_copy(out=k_sb[:, tsl], in_=qps[0:D, :])
                    nc.scalar.activation(out=q_hi[D:2 * D, tsl],
                                         in_=qps[D:2 * D, :],
                                         func=AF.Identity, bias=bq_hi[:],
                                         scale=1.0)
                    nc.sync.dma_start(out=q_sb[:, tsl],
                                      in_=q_hi[D:2 * D, tsl])

                # ---- attention ----
                et = {}

                def emit_st2(tc_i, pr):
                    # two fp16 S^T matmuls back to back (one bf16<->fp16
                    # dtype switch per pair instead of per matmul), one
                    # 1024-wide exp
                    tsl = slice(tc_i * TW, (tc_i + 1) * TW)
                    stp = ps.tile([128, 2 * TW], F32,
                                  name=f"st_{b}_{tc_i}_{pr}", tag="stp")
                    for h in range(2):
                        sc = 2 * pr + h
                        nc.tensor.matmul(
                            stp[:, h * TW:(h + 1) * TW],
                            k_sb[:, sc * 128:(sc + 1) * 128],
                            q_sb[:, tsl], start=True, stop=True)
                    t_et = etp.tile([128, 2 * TW], BF16,
                                    name=f"et_{b}_{tc_i}_{pr}", tag=f"et{pr}")
                    for h in range(2):
                        hs = slice(h * TW, (h + 1) * TW)
                        nc.scalar.activation(out=t_et[:, hs], in_=stp[:, hs],
                                             func=AF.Exp)
                    et[(tc_i, pr)] = t_et

                emit_st2(0, 0)
                for tc_i in range(TCH):
                    tsl = slice(tc_i * TW, (tc_i + 1) * TW)
                    den = ps.tile([128, TW], F32, name=f"den_{b}_{tc_i}",
                                  tag="den", bufs=2)
                    oacc = [ps.tile([128, TW], F32, name=f"o_{b}_{tc_i}_{cc}",
                                    tag=f"o{cc}") for cc in range(CCH)]
                    # residual (+ gamma*bv) prefetch for this chunk
                    xg_sb = []
                    for cc in range(CCH):
                        t_xg = finp.tile([128, TW], F32,
                                         name=f"xg_{b}_{tc_i}_{cc}", tag="xg",
                                         bufs=6)
                        nc.sync.dma_start(
                            out=t_xg,
                            in_=xgd[b, cc * 128:(cc + 1) * 128, tsl])
                        xg_sb.append(t_xg)

                    NPR = SCH // 2
                    for pr in range(NPR):
                        # next pair's S^T/exp first so ACT keeps the PE fed
                        if pr + 1 < NPR:
                            emit_st2(tc_i, pr + 1)
                        elif tc_i + 1 < TCH:
                            emit_st2(tc_i + 1, 0)
                        e = et.pop((tc_i, pr))
                        # sum the two halves on DVE so den needs one matmul
                        # per pair instead of two
                        e2 = etp.tile([128, TW], BF16,
                                      name=f"e2_{b}_{tc_i}_{pr}", tag="e2",
                                      bufs=3)
                        nc.gpsimd.tensor_add(e2[:], e[:, 0:TW], e[:, TW:2 * TW])
                        for h in range(2):
                            sc = 2 * pr + h
                            esl = e[:, h * TW:(h + 1) * TW]
                            for cc in range(CCH):
                                nc.tensor.matmul(
                                    oacc[cc][:],
                                    vt_sb[sc][:, cc * 128:(cc + 1) * 128],
                                    esl, start=(sc == 0),
                                    stop=(sc == SCH - 1))
                        nc.tensor.matmul(den[:], ones[:], e2[:],
                                         start=(pr == 0),
                                         stop=(pr == NPR - 1))

                    # free o/den PSUM banks via ACT copies; slow DVE recip
                    # runs off the PE critical path
                    recip = finp.tile([128, TW], F32,
                                      name=f"rc_{b}_{tc_i}", tag="rc", bufs=2)
                    nc.vector.reciprocal(out=recip[:], in_=den[:])
                    last = (b == BPC - 1 and tc_i == TCH - 1)
                    for cc in range(CCH):
                        if last:
                            # tail: o-banks are not needed soon, skip the
                            # bank-freeing copy and read PSUM directly
                            o_src = oacc[cc][:]
                        else:
                            t_o = finp.tile([128, TW], F32,
                                            name=f"ob_{b}_{tc_i}_{cc}",
                                            tag=f"ob{cc}", bufs=2)
                            nc.scalar.activation(out=t_o[:], in_=oacc[cc][:],
                                                 func=AF.Copy)
                            o_src = t_o[:]
                        t_f = finp.tile([128, TW], F32,
                                        name=f"f_{b}_{tc_i}_{cc}", tag="f",
                                        bufs=3)
                        nc.vector.tensor_mul(t_f[:], o_src, recip[:])
                        nc.vector.tensor_add(t_f[:], t_f[:], xg_sb[cc][:])
                        nc.sync.dma_start(
                            out=outd[b, cc * 128:(cc + 1) * 128, tsl],
                            in_=t_f)
    nc.compile()
    return nc


def _get_nc():
    if "nc" not in _CACHE:
        _CACHE["nc"] = _build()
    return _CACHE["nc"]


def kernel(x, wq, bq, wk, bk, wv, bv, gamma):
    global LAST_EXEC_NS
    g = float(np.asarray(gamma).reshape(-1)[0])
    x = np.asarray(x, np.float32)
    # fold gamma into the v path; bk cancels inside softmax; the v bias
    # contributes gamma*bv per channel (softmax rows sum to 1) -> fold it
    # plus the residual into xg
    wvT = np.ascontiguousarray((g * np.asarray(wv, np.float32)).T).astype(np.float16)
    wkqT = np.concatenate([np.asarray(wk, np.float32).T,
                           np.asarray(wq, np.float32).T],
                          axis=1).astype(np.float16)
    bq2 = np.asarray(bq, np.float32).reshape(D, 1)
    gbv = (g * np.asarray(bv, np.float32)).reshape(1, C, 1)
    xg = x + gbv
    ones = np.ones((128, 128), ml_dtypes.bfloat16)
    xh = x.astype(np.float16)

    in_maps = []
    for core in range(NCORES):
        sl = slice(core * BPC, (core + 1) * BPC)
        in_maps.append({
            "x": xh[sl], "xg": xg[sl],
            "wkqT": wkqT, "wvT": wvT,
            "bq": bq2, "ones": ones,
        })

    nc = _get_nc()
    res = run_bass_kernel_spmd(nc, in_maps, core_ids=list(range(NCORES)),
                               trace=PROFILE)
    LAST_EXEC_NS = res.exec_time_ns
    out = np.empty((B, C, T), np.float32)
    for core in range(NCORES):
        out[core * BPC:(core + 1) * BPC] = res.results[core]["out"]
    return out


# revision 10
# speedup vs baseline: 1.0185x; 1.0185x over previous
"""Trainium2 Bass kernel for an attention block (B=16, C=512, T=2048).

reference:
  q = wq@x + bq; k = wk@x + bk; v = wv@x + bv          (conv1x1 per sample)
  attn = softmax(q^T k over s); out = v @ attn^T
  result = gamma * out + x

Sharding: data-parallel over batch across 8 NeuronCores (2 samples/core),
weights replicated.

Device algorithm (per sample):
  - host folds gamma into wv, and gamma*bv + x into the residual xg
    (softmax rows sum to 1, so the v-bias is a per-channel constant);
    bk is dropped (a per-t constant in scores cancels in softmax over s).
  - q/k/scores path in fp16 (1 PE cycle/row, 11-bit mantissa incl
    implicit); v/softmax-weights path in bf16 (range: exp(S) up to e^64);
    PSUM accumulation always fp32.
  - v^T[s,o] tiles via matmul(lhsT=x_fp16[c,s], rhs=(gamma*wv)^T[c,o])
  - q[d,t], k[d,s] via matmul(lhsT=wq^T/wk^T, rhs=x_fp16); bias only on q
  - per 512-wide t-chunk, for each 128-wide s-chunk (sw pipelined by 2,
    next chunk's first two S^T/exp pairs pre-emitted so the PE never
    starves across chunk boundaries -> HAM stays at K=8/8):
      S^T[s,t] = matmul(lhsT=k[:,s], rhs=q[:,t])      (fp16, K=64, N=512)
      E = exp(S^T)  (ACT, PSUM->SBUF bf16; no max-subtraction: |S|<~64)
      den += matmul(lhsT=ones128, rhs=E)              (bf16; sum over s,
                                                       broadcast on parts)
      out0[c,t] += matmul(lhsT=v^T[s,c], rhs=E)       (bf16; 4 c-chunks)
    then: out0 -> SBUF (ACT, frees PSUM), r = recip(den) (DVE),
          result = out0 * r + xg (DVE mul+add) -> DMA out
"""
import numpy as np
import ml_dtypes
import concourse.bass as bass
import concourse.bacc as bacc
import concourse.tile as tile
from concourse import mybir
from concourse.bass_utils import run_bass_kernel_spmd

F32 = mybir.dt.float32
FP16 = mybir.dt.float16
BF16 = mybir.dt.bfloat16
AF = mybir.ActivationFunctionType

B, C, T, D = 16, 512, 2048, 64
NCORES = 8
BPC = B // NCORES          # samples per core
CCH = C // 128             # 4 channel chunks
TW = 512                   # t tile width (matmul free dim)
TCH = T // TW              # 4 t chunks
SCH = T // 128             # 16 s chunks

PROFILE = False            # set True before calling kernel() to capture HW time
LAST_EXEC_NS = None
_CACHE = {}


def _round_fp32r(a: np.ndarray) -> np.ndarray:
    """Round fp32 to fp32r precision (11 explicit mantissa bits, RNE)."""
    u = np.ascontiguousarray(a, dtype=np.float32).view(np.uint32)
    lsb = (u >> 12) & 1
    rounded = u + np.uint32(0x7FF) + lsb
    return (rounded & np.uint32(0xFFFFF000)).astype(np.uint32).view(np.float32)


def _build():
    nc = bacc.Bacc("TRN2", target_bir_lowering=False, debug=False,
                   enable_asserts=False)
    xd = nc.dram_tensor("x", [BPC, C, T], FP16, kind="ExternalInput").ap()
    xgd = nc.dram_tensor("xg", [BPC, C, T], F32, kind="ExternalInput").ap()
    wkqT = nc.dram_tensor("wkqT", [C, 2 * D], FP16, kind="ExternalInput").ap()
    wvT = nc.dram_tensor("wvT", [C, C], FP16, kind="ExternalInput").ap()
    bqd = nc.dram_tensor("bq", [D, 1], F32, kind="ExternalInput").ap()
    onesd = nc.dram_tensor("ones", [128, 128], BF16, kind="ExternalInput").ap()
    outd = nc.dram_tensor("out", [BPC, C, T], F32, kind="ExternalOutput").ap()

    with tile.TileContext(nc) as tc:
        with tc.tile_pool(name="const", bufs=1) as constp, \
             tc.tile_pool(name="xp", bufs=2) as xp, \
             tc.tile_pool(name="vtp", bufs=1) as vtp, \
             tc.tile_pool(name="qkp", bufs=1) as qkp, \
             tc.tile_pool(name="etp", bufs=1) as etp, \
             tc.tile_pool(name="finp", bufs=1) as finp, \
             tc.tile_pool(name="ps", bufs=1, space="PSUM") as ps:

            # ---- x as [128, CCH, T] (one DMA per quarter; dma_start
            #      issue on the sync queue costs ~0.6us each, so batch) ----
            x_big_all = [xp.tile([128, CCH, T], FP16, name=f"x_{b}", tag="x")
                         for b in range(BPC)]

            def load_x(b, q4):
                qsl = slice(q4 * TW, (q4 + 1) * TW)
                nc.sync.dma_start(
                    out=x_big_all[b][:, :, qsl],
                    in_=xd[b, :, qsl].rearrange("(c p) t -> p c t", p=128))

            # wv first (first matmul needs it), then the first x quarter,
            # then the rest
            wv_big = constp.tile([128, CCH, C], FP16)
            nc.sync.dma_start(
                out=wv_big,
                in_=wvT.rearrange("(c p) o -> p c o", p=128))
            load_x(0, 0)
            wkq_big = constp.tile([128, CCH, 2 * D], FP16)
            nc.sync.dma_start(
                out=wkq_big,
                in_=wkqT.rearrange("(c p) d -> p c d", p=128))
            ones = constp.tile([128, 128], BF16)
            nc.sync.dma_start(out=ones, in_=onesd)
            bq_full = constp.tile([128, 1], F32)
            nc.sync.dma_start(out=bq_full[D:2 * D, :], in_=bqd)
            bq_hi = bq_full[D:2 * D, :]
            for q4 in range(1, 4):
                load_x(0, q4)
            wv_sb = [wv_big[:, cc, :] for cc in range(CCH)]
            wq_sb = [wkq_big[:, cc, :] for cc in range(CCH)]

            for b in range(BPC):
                x_big = x_big_all[b]
                x_sb = [x_big[:, cc, :] for cc in range(CCH)]
                if b > 0:
                    for q4 in range(4):
                        load_x(b, q4)

                # ---- v^T tiles (bf16): vt[sc][s=128, o=512] ----
                vt_sb = []
                for sc in range(SCH):
                    vps = ps.tile([128, TW], F32, name=f"vps_{b}_{sc}",
                                  tag=f"o{sc % 2}")
                    for cc in range(CCH):
                        nc.tensor.matmul(
                            vps[:],
                            x_sb[cc][:, sc * 128:(sc + 1) * 128],
                            wv_sb[cc][:],
                            start=(cc == 0), stop=(cc == CCH - 1))
                    t_vt = vtp.tile([128, C], BF16, name=f"vt_{b}_{sc}",
                                    tag=f"vt{sc}")
                    nc.vector.tensor_copy(out=t_vt[:], in_=vps[:])
                    vt_sb.append(t_vt)

                # ---- q, k via one M=128 matmul (k rows 0:64, q rows
                #      64:128); q then shifted to partitions 0:64 by DMA so
                #      the S^T matmul operands share a partition range ----
                q_hi = qkp.tile([128, T], FP16, name=f"qh_{b}", tag="qh")
                q_sb = qkp.tile([D, T], FP16, name=f"q_{b}", tag="q")
                k_sb = qkp.tile([D, T], FP16, name=f"k_{b}", tag="k")
                for tc_i in range(TCH):
                    tsl = slice(tc_i * TW, (tc_i + 1) * TW)
                    qps = ps.tile([128, TW], F32, name=f"qps_{b}_{tc_i}",
                                  tag=f"o{2 + tc_i % 2}")
                    for cc in range(CCH):
                        nc.tensor.matmul(qps[:], wq_sb[cc][:],
                                         x_sb[cc][:, tsl],
                                         start=(cc == 0), stop=(cc == CCH - 1))
                    nc.vector.tensor_copy(out=k_sb[:, tsl], in_=qps[0:D, :])
                    nc.scalar.activation(out=q_hi[D:2 * D, tsl],
                                         in_=qps[D:2 * D, :],
                                         func=AF.Identity, bias=bq_hi[:],
                                         scale=1.0)
                    nc.sync.dma_start(out=q_sb[:, tsl],
                                      in_=q_hi[D:2 * D, tsl])

                # ---- attention ----
                et = {}

                def emit_st2(tc_i, pr):
                    # two fp16 S^T matmuls back to back (one bf16<->fp16
                    # dtype switch per pair instead of per matmul), one
                    # 1024-wide exp
                    tsl = slice(tc_i * TW, (tc_i + 1) * TW)
                    stp = ps.tile([128, 2 * TW], F32,
                                  name=f"st_{b}_{tc_i}_{pr}", tag="stp")
                    for h in range(2):
                        sc = 2 * pr + h
                        nc.tensor.matmul(
                            stp[:, h * TW:(h + 1) * TW],
                            k_sb[:, sc * 128:(sc + 1) * 128],
                            q_sb[:, tsl], start=True, stop=True)
                    t_et = etp.tile([128, 2 * TW], BF16,
                                    name=f"et_{b}_{tc_i}_{pr}", tag=f"et{pr}")
                    for h in range(2):
                        hs = slice(h * TW, (h + 1) * TW)
                        nc.scalar.activation(out=t_et[:, hs], in_=stp[:, hs],
                                             func=AF.Exp)
                    et[(tc_i, pr)] = t_et

                emit_st2(0, 0)
                for tc_i in range(TCH):
                    tsl = slice(tc_i * TW, (tc_i + 1) * TW)
                    den = ps.tile([128, TW], F32, name=f"den_{b}_{tc_i}",
                                  tag="den", bufs=2)
                    oacc = [ps.tile([128, TW], F32, name=f"o_{b}_{tc_i}_{cc}",
                                    tag=f"o{cc}") for cc in range(CCH)]
                    # residual (+ gamma*bv) prefetch for this chunk
                    xg_t = finp.tile([128, CCH, TW], F32,
                                     name=f"xg_{b}_{tc_i}", tag="xg", bufs=3)
                    nc.sync.dma_start(
                        out=xg_t,
                        in_=xgd[b, :, tsl].rearrange("(c p) t -> p c t",
                                                     p=128))
                    xg_sb = [xg_t[:, cc, :] for cc in range(CCH)]

                    NPR = SCH // 2
                    for pr in range(NPR):
                        # next pair's S^T/exp first so ACT keeps the PE fed
                        if pr + 1 < NPR:
                            emit_st2(tc_i, pr + 1)
                        elif tc_i + 1 < TCH:
                            emit_st2(tc_i + 1, 0)
                        e = et.pop((tc_i, pr))
                        # sum the two halves on DVE so den needs one matmul
                        # per pair instead of two
                        e2 = etp.tile([128, TW], BF16,
                                      name=f"e2_{b}_{tc_i}_{pr}", tag="e2",
                                      bufs=3)
                        nc.gpsimd.tensor_add(e2[:], e[:, 0:TW], e[:, TW:2 * TW])
                        for h in range(2):
                            sc = 2 * pr + h
                            esl = e[:, h * TW:(h + 1) * TW]
                            for cc in range(CCH):
                                nc.tensor.matmul(
                                    oacc[cc][:],
                                    vt_sb[sc][:, cc * 128:(cc + 1) * 128],
                                    esl, start=(sc == 0),
                                    stop=(sc == SCH - 1))
                        nc.tensor.matmul(den[:], ones[:], e2[:],
                                         start=(pr == 0),
                                         stop=(pr == NPR - 1))

                    # free o/den PSUM banks via ACT copies; slow DVE recip
                    # runs off the PE critical path
                    recip = finp.tile([128, TW], F32,
                                      name=f"rc_{b}_{tc_i}", tag="rc", bufs=2)
                    nc.vector.reciprocal(out=recip[:], in_=den[:])
                    last = (b == BPC - 1 and tc_i == TCH - 1)
                    t_f = finp.tile([128, CCH, TW], F32,
                                    name=f"f_{b}_{tc_i}", tag="f", bufs=2)
                    for cc in range(CCH):
                        if last:
                            # tail: o-banks are not needed soon, skip the
                            # bank-freeing copy and read PSUM directly
                            o_src = oacc[cc][:]
                        else:
                            t_o = finp.tile([128, TW], F32,
                                            name=f"ob_{b}_{tc_i}_{cc}",
                                            tag=f"ob{cc}", bufs=2)
                            nc.scalar.activation(out=t_o[:], in_=oacc[cc][:],
                                                 func=AF.Copy)
                            o_src = t_o[:]
                        nc.vector.tensor_mul(t_f[:, cc, :], o_src, recip[:])
                        nc.vector.tensor_add(t_f[:, cc, :], t_f[:, cc, :],
                                             xg_sb[cc][:])
                    nc.sync.dma_start(
                        out=outd[b, :, tsl].rearrange("(c p) t -> p c t",
                                                      p=128),
                        in_=t_f)
    nc.compile()
    return nc


def _get_nc():
    if "nc" not in _CACHE:
        _CACHE["nc"] = _build()
    return _CACHE["nc"]


def kernel(x, wq, bq, wk, bk, wv, bv, gamma):
    global LAST_EXEC_NS
    g = float(np.asarray(gamma).reshape(-1)[0])
    x = np.asarray(x, np.float32)
    # fold gamma into the v path; bk cancels inside softmax; the v bias
    # contributes gamma*bv per channel (softmax rows sum to 1) -> fold it
    # plus the residual into xg
    wvT = np.ascontiguousarray((g * np.asarray(wv, np.float32)).T).astype(np.float16)
    wkqT = np.concatenate([np.asarray(wk, np.float32).T,
                           np.asarray(wq, np.float32).T],
                          axis=1).astype(np.float16)
    bq2 = np.asarray(bq, np.float32).reshape(D, 1)
    gbv = (g * np.asarray(bv, np.float32)).reshape(1, C, 1)
    xg = x + gbv
    ones = np.ones((128, 128), ml_dtypes.bfloat16)
    xh = x.astype(np.float16)

    in_maps = []
    for core in range(NCORES):
        sl = slice(core * BPC, (core + 1) * BPC)
        in_maps.append({
            "x": xh[sl], "xg": xg[sl],
            "wkqT": wkqT, "wvT": wvT,
            "bq": bq2, "ones": ones,
        })

    nc = _get_nc()
    res = run_bass_kernel_spmd(nc, in_maps, core_ids=list(range(NCORES)),
                               trace=PROFILE)
    LAST_EXEC_NS = res.exec_time_ns
    out = np.empty((B, C, T), np.float32)
    for core in range(NCORES):
        out[core * BPC:(core + 1) * BPC] = res.results[core]["out"]
    return out
